# revision 43
# baseline (speedup 1.0000x reference)
"""Trainium2 Bass kernel for nn_AttentionHead (B=4, S=4096, H=1024, D=64).

Reference computation (note the unusual K-first ordering):
    K = x @ Wk.T; Q = x @ Wq.T; V = x @ Wv.T            [B,S,D]
    scores[b,i,j] = (K[b,i] . Q[b,j]) / sqrt(D)         [B,S,S]
    scores[:, :, j] = -1e12 where mask[:, j] == 0
    out = softmax(scores, axis=2) @ V                   [B,S,S] @ [B,S,D]

Key structural choices:
  - Masked j-columns get softmax weight EXACTLY 0 (exp underflows), so the
    host drops them up front: the query/value axis is compacted from the
    mask (~2048 of 4096 survive) and padded to J = ceil(max_keep/128)*128
    (the build is parameterized on J and on the count of fully-kept tiles,
    so any mask density still compiles a correct program). This halves the
    scores/exp/AV work, which dominates.
  - x^T in bf16 is pure data movement, so the host ships it pre-transposed
    (like the baseline's host-side roll): no on-chip transposes of x, no
    fp32->bf16 casts, and half the HBM traffic. Weights/identities are
    host-cast too, so no DMA needs the (slow, gpsimd-only) cast path and
    bulk input streams across all three DMA trigger queues (gpsimd SWDGE +
    sync/scalar HWDGE).
  - Scores use PE row tiling: contraction is only D=64, so the two 512-wide
    score matmuls of a slot run CONCURRENTLY on row groups 0/1 of the PE
    array (~2x on the scores leg, and their LDWEIGHTS overlap in-flight
    matmuls of the other group). This requires Q^T and K^T duplicated into
    partitions 64:128: K^T comes for free from a [Wk|Wk] stationary
    projection; Q^T via one DVE partition-shift copy per block.

Sharding: 8 cores = 4 batches x 2 key-row halves of 2048. Each core gets
x^T for its own 2048 key rows (xtk) plus the batch-shared mask-compacted
x^T for queries/values (xtq).

Per-core pipeline (bf16 matmuls, fp32 accumulation):
  - One [Wq|Wv] stationary gives Q^T (rows 0:64) and V^T (rows 64:128) per
    query-column block; [Wk|Wk] gives duplicated K^T over own 2048 rows;
    V^T -> V via PE transposes. V gets a ones column (softmax denominator).
  - PE warmup matmuls on junk data cover the DMA ramp so the HAM
    clock-gate sits at 8/8 when real work arrives.
  - Two passes over query tiles t=0..JT-1 (one per 1024-wide i-half). Per
    slot: scores^T = Q^T_t.T @ K^T on PE (row-tiled pair); exp(0.125*s +
    maskbias[j]) on ACT (mask/pad folded into the per-partition bias;
    masked queries underflow to exactly 0); PE accumulates V'_t.T @ P^T_t
    into out'^T [65, 1024] - rows 0:64 numerator^T, row 64 denominator.
    The AV matmuls are emitted one slot BEHIND the scores matmuls:
    otherwise they head-of-line block the PE queue waiting on exp. Pass A
    is emission-interleaved with the projection stream so PE always has
    dense work chasing the DMA; pass-B-only data (xtk cols 1024:2048) is
    deprioritized in the DMA queues.
  - Per-pass finale: copy acc to SBUF (freeing the PSUM accumulator), then
    128-col pieces: PE-transpose via identity matmul, out = numerator *
    reciprocal(denominator) on DVE, store every 256 rows as soon as ready.
    Pass A's finale pieces are interleaved into pass B's early slots where
    the PE has slack (pass B is exp/ACT-paced).
"""

import numpy as np

B, S, H, D = 4, 4096, 1024, 64
N_CORES = 8
SC = S // 2  # key rows (output rows) per core
HC = H // 128  # contraction chunks
J_MIN = 1024  # floor for the padded, mask-compacted query-column count
NEG = -30000.0
N_WARM = 40

_CACHE = {}


def _build(J, nfull):
    # nfull: query tiles [0, nfull) are fully kept for EVERY batch, so their
    # exp needs no mask bias (saves the ACT per-partition bias read).
    import concourse.tile as tile
    from concourse import bacc, mybir

    dt = mybir.dt
    AF = mybir.ActivationFunctionType
    JT = J // 128
    qblocks = [(c0, min(c0 + 512, J)) for c0 in range(0, J, 512)]

    nc = bacc.Bacc(
        "TRN2", target_bir_lowering=False, debug=False, num_devices=N_CORES
    )
    xtk = nc.dram_tensor("xtk", [H, SC], dt.bfloat16, kind="ExternalInput").ap()
    xtq = nc.dram_tensor("xtq", [H, J], dt.bfloat16, kind="ExternalInput").ap()
    wqv = nc.dram_tensor("wqv", [H, 2 * D], dt.bfloat16, kind="ExternalInput").ap()
    wkk = nc.dram_tensor("wkk", [H, 2 * D], dt.bfloat16, kind="ExternalInput").ap()
    mb = nc.dram_tensor("mb", [128, JT], dt.float32, kind="ExternalInput").ap()
    idb = nc.dram_tensor("idb", [128, 128], dt.bfloat16, kind="ExternalInput").ap()
    idf = nc.dram_tensor("idf", [D + 1, D + 1], dt.float32, kind="ExternalInput").ap()
    out = nc.dram_tensor("out", [SC, D], dt.float32, kind="ExternalOutput").ap()

    xtk_r = xtk.rearrange("(c p) s -> p c s", p=128)
    xtq_r = xtq.rearrange("(c p) s -> p c s", p=128)

    with (
        tile.TileContext(nc) as tc,
        tc.tile_pool(name="persist", bufs=1) as persist,
        tc.tile_pool(name="ptile", bufs=6) as ptile,
        tc.tile_pool(name="accs", bufs=2) as accs,
        tc.tile_pool(name="fin", bufs=2) as fin,
    ):
        qt = persist.tile([128, J], dt.bfloat16)  # Q^T duplicated rows 0:64/64:128
        kt = persist.tile([128, SC], dt.bfloat16)  # K^T duplicated rows 0:64/64:128
        vtsb = persist.tile([128, J], dt.bfloat16)  # rows 64:128 = V^T
        vp = persist.tile([128, JT, D + 1], dt.bfloat16)
        mb_sb = persist.tile([128, JT], dt.float32)
        idf_sb = persist.tile([D + 1, D + 1], dt.float32)
        idb_sb = persist.tile([128, 128], dt.bfloat16)
        wtile = persist.tile([128, 512], dt.bfloat16)
        xk_sb = persist.tile([128, HC, SC], dt.bfloat16)
        xq_sb = persist.tile([128, HC, J], dt.bfloat16)
        wqv_sb = persist.tile([128, HC, 2 * D], dt.bfloat16)
        wkk_sb = persist.tile([128, HC, 2 * D], dt.bfloat16)

        nc.vector.memset(vp[:, :, D], 1.0)
        nc.vector.memset(wtile[:], 0.0)

        with (
            tc.tile_pool(name="psco", bufs=2, space="PSUM") as psco,
            tc.tile_pool(name="ppx", bufs=2, space="PSUM") as ppx,
            tc.tile_pool(name="pacc", bufs=1, space="PSUM") as pacc,
        ):
            # --- DMA queue plans; pass-A-critical data first on each queue ---
            def big_loads():
                # The 3 trigger queues split HBM bandwidth roughly evenly and
                # each queue serializes its own transfers, so the FIRST items
                # of every queue are what arrive early. kt's prerequisites
                # (wkk + xtk cols 0:1024) are striped across all three queues
                # first, each unit a [128, 4, 512] block with 1KB contiguous
                # lines. The scalar (ACT) queue gets only early items so exp
                # is never stuck behind a stalled DMA trigger. Pass-B-only
                # xtk cols 1024:2048 go last.
                # All three trigger engines pump concurrently (~1/3 of HBM
                # bandwidth each); units are [128, 4, 512] H-chunk halves
                # (1KB contiguous lines) round-robined in global priority
                # order. Pass-B-only xtk cols 1024:2048 go last.
                nc.gpsimd.dma_start(
                    wkk_sb[:], wkk.rearrange("(c p) d -> p c d", p=128)
                )
                nc.scalar.dma_start(
                    wqv_sb[:], wqv.rearrange("(c p) d -> p c d", p=128)
                )
                nc.sync.dma_start(mb_sb[:], mb[:])
                nc.sync.dma_start(idf_sb[:], idf[:])
                nc.sync.dma_start(idb_sb[:], idb[:])
                # All three trigger engines pump concurrently (~1/3 of HBM
                # bandwidth each); 0.5MB units with 1KB contiguous lines,
                # round-robined in global priority order (kt first, pass-B
                # -only xtk cols 1024:2048 last). Measured best overall
                # despite exp(0) waiting out the scalar queue's share.
                qs = [nc.sync, nc.gpsimd, nc.scalar]
                qi = 0

                def unit(dst, src, c0, c1):
                    nonlocal qi
                    for h0, h1 in ((0, 4), (4, 8)):
                        qs[qi % 3].dma_start(
                            dst[:, h0:h1, c0:c1], src[:, h0:h1, c0:c1]
                        )
                        qi += 1

                for c in range(0, 1024, 512):
                    unit(xk_sb, xtk_r, c, c + 512)
                for c in range(0, J, 512):
                    unit(xq_sb, xtq_r, c, min(c + 512, J))
                for c in range(1024, 2048, 512):
                    unit(xk_sb, xtk_r, c, c + 512)

            # --- PE work generators ---
            def proj_qv(bi):  # [Q^T; V^T] for one query-column block
                c0, c1 = qblocks[bi]
                ps = ppx.tile([128, c1 - c0], dt.float32, tag="px")
                for hc in range(HC):
                    nc.tensor.matmul(
                        ps[:],
                        wqv_sb[:, hc, :],
                        xq_sb[:, hc, c0:c1],
                        start=(hc == 0),
                        stop=(hc == HC - 1),
                    )
                # slot scores need qt + its dup first; vtsb (-> VT -> AV)
                # has a slot of slack
                nc.vector.tensor_copy(qt[0:64, c0:c1], ps[0:64, :])
                nc.vector.tensor_copy(qt[64:128, c0:c1], ps[0:64, :])
                nc.vector.tensor_copy(vtsb[64:128, c0:c1], ps[64:128, :])

            def proj_k(sb):  # [Wk|Wk] stationary -> K^T in both halves
                ps = ppx.tile([128, 512], dt.float32, tag="px")
                for hc in range(HC):
                    nc.tensor.matmul(
                        ps[:],
                        wkk_sb[:, hc, :],
                        xk_sb[:, hc, 512 * sb : 512 * (sb + 1)],
                        start=(hc == 0),
                        stop=(hc == HC - 1),
                    )
                nc.vector.tensor_copy(kt[:, 512 * sb : 512 * (sb + 1)], ps[:])

            def vt_block(st0, st1):  # V^T -> V via PE transpose
                for st in range(st0, st1):
                    pvt = ppx.tile([128, D], dt.bfloat16, tag="px")
                    nc.tensor.transpose(
                        pvt[:],
                        vtsb[64:128, 128 * st : 128 * (st + 1)],
                        idb_sb[64:128, 64:128],
                    )
                    nc.vector.tensor_copy(vp[:, st, 0:D], pvt[:])

            # --- t-loop slot machinery: AV deferred one slot behind (the
            # pending entry carries its own accumulator, so the deferral
            # crosses the pass A -> pass B boundary without serializing) ---
            pending = []

            def flush_av():
                if not pending:
                    return
                pt, t, acc = pending.pop()
                for nb in range(2):
                    nc.tensor.matmul(
                        acc[:, 512 * nb : 512 * (nb + 1)],
                        vp[:, t, :],
                        pt[:, 512 * nb : 512 * (nb + 1)],
                        start=(t == 0),
                        stop=(t == JT - 1),
                    )

            def t_slot(t, acc, ih):
                ps = psco.tile([128, 1024], dt.float32, tag="ps")
                # row-tiled pair: groups 0/1 run concurrently (contraction 64)
                nc.tensor.matmul(
                    ps[:, 0:512],
                    qt[0:64, 128 * t : 128 * (t + 1)],
                    kt[0:64, 1024 * ih : 1024 * ih + 512],
                    start=True,
                    stop=True,
                )
                nc.tensor.matmul(
                    ps[:, 512:1024],
                    qt[64:128, 128 * t : 128 * (t + 1)],
                    kt[64:128, 1024 * ih + 512 : 1024 * ih + 1024],
                    start=True,
                    stop=True,
                )
                flush_av()
                pt = ptile.tile([128, 1024], dt.bfloat16)
                if t < nfull:
                    nc.scalar.activation(pt[:], ps[:], AF.Exp, scale=0.125)
                elif t == JT - 1:
                    # last slot: exp in halves so the final AV (and with it
                    # the finale chain) starts half a slot earlier
                    for nb in range(2):
                        sl = slice(512 * nb, 512 * (nb + 1))
                        nc.scalar.activation(
                            pt[:, sl], ps[:, sl], AF.Exp,
                            bias=mb_sb[:, t : t + 1], scale=0.125,
                        )
                else:
                    nc.scalar.activation(
                        pt[:], ps[:], AF.Exp, bias=mb_sb[:, t : t + 1], scale=0.125
                    )
                pending.append((pt, t, acc))

            def acc_to_sb(acc):  # copy PSUM acc to SBUF, freeing pacc
                acc_sb = accs.tile([D + 1, 1024], dt.float32, tag="accs")
                for c in range(0, 1024, 256):  # 256-wide: finale piece 0
                    nc.vector.tensor_copy(  # starts after the first chunk
                        acc_sb[:, c : c + 256], acc[:, c : c + 256]
                    )
                return acc_sb

            def finale_piece(acc_sb, ih, k):  # one 128-col chunk
                po = ppx.tile([128, D + 1], dt.float32, tag="px")
                nc.tensor.transpose(
                    po[:], acc_sb[:, 128 * k : 128 * (k + 1)], idf_sb[:]
                )
                rc = fin.tile([128, 1], dt.float32, tag="rc")
                nc.vector.reciprocal(rc[:], po[:, D : D + 1])
                nc.vector.tensor_scalar_mul(
                    oall[:, 8 * ih + k, :], po[:, 0:D], rc[:]
                )
                if k % 2 == 1:  # store every 256 rows as soon as ready
                    r0 = 1024 * ih + 128 * (k - 1)
                    nc.sync.dma_start(
                        out[r0 : r0 + 256, :].rearrange("(k p) d -> p k d", p=128),
                        oall[:, 8 * ih + k - 1 : 8 * ih + k + 1, :],
                    )

            oall = fin.tile([128, 16, D], dt.float32, tag="oall")

            # ---- pass A (i-half 0) interleaved with the projections ----
            big_loads()
            accA = pacc.tile([D + 1, 1024], dt.float32, tag="acc")
            tA = lambda t: t_slot(t, accA, 0)
            # PE warmup while the first slices stream in
            pw = ppx.tile([128, 512], dt.float32, tag="px")
            for _ in range(N_WARM):
                nc.tensor.matmul(
                    pw[:], wtile[:, 0:128], wtile[:], start=True, stop=True
                )
            dummy = fin.tile([128, 1], dt.float32, tag="dummy")
            nc.scalar.activation(dummy[:], wtile[:, 0:1], AF.Exp)
            proj_k(0)
            proj_k(1)
            proj_qv(0)
            vt_cover = qblocks[0][1] // 128
            vt_block(0, vt_cover)
            next_t = 0
            units = [("qv", i) for i in range(1, len(qblocks))]
            units += [("k", 2), ("k", 3)]
            for kind, i in units:
                # emit already-runnable slots BEFORE the next proj unit:
                # the PE queue is in-order, so a proj waiting on its DMA
                # must not head-of-line-block ready slots
                tgt = min(vt_cover, next_t + 2)
                while next_t < tgt:
                    tA(next_t)
                    next_t += 1
                if kind == "qv":
                    proj_qv(i)
                    new_cover = qblocks[i][1] // 128
                    vt_block(vt_cover, new_cover)
                    vt_cover = new_cover
                else:
                    proj_k(i)
            while next_t < JT:
                tA(next_t)
                next_t += 1
            # A's last AV stays pending: it flushes inside pass B's slot 0,
            # so pass-B scores aren't serialized behind it

            # ---- pass B (i-half 1), finale A interleaved into its slack ----
            accB = pacc.tile([D + 1, 1024], dt.float32, tag="acc")
            fa = 0
            acc_sbA = None
            for t in range(JT):
                t_slot(t, accB, 1)
                if t == 0:
                    acc_sbA = acc_to_sb(accA)  # frees pacc banks for accB
                elif fa < 8:
                    finale_piece(acc_sbA, 0, fa)
                    fa += 1
            while fa < 8:
                finale_piece(acc_sbA, 0, fa)
                fa += 1
            flush_av()
            acc_sbB = acc_to_sb(accB)
            for k in range(8):
                finale_piece(acc_sbB, 1, k)

    nc.compile()
    return nc


def _in_maps(x, mask, Wk, Wq, Wv):
    import ml_dtypes

    bf16 = ml_dtypes.bfloat16
    wqv = np.ascontiguousarray(
        np.concatenate([Wq.T, Wv.T], axis=1).astype(bf16)
    )
    wkk = np.ascontiguousarray(np.concatenate([Wk.T, Wk.T], axis=1).astype(bf16))
    idb = np.eye(128, dtype=bf16)
    idf = np.eye(D + 1, dtype=np.float32)
    nk = [int((mask[b] != 0).sum()) for b in range(B)]
    J = max(J_MIN, -(-max(nk) // 128) * 128)
    nfull = min(nk) // 128
    JT = J // 128
    xtq_b, mb_b = [], []
    for b in range(B):
        idx = np.flatnonzero(mask[b] != 0)
        xt = np.zeros((H, J), dtype=bf16)
        xt[:, : len(idx)] = x[b].T[:, idx].astype(bf16)
        xtq_b.append(xt)
        mbv = np.full(J, np.float32(NEG), dtype=np.float32)
        mbv[: len(idx)] = 0.0
        mb_b.append(np.ascontiguousarray(mbv.reshape(JT, 128).T))
    maps = []
    for c in range(N_CORES):
        b, half = c // 2, c % 2
        xtk = np.ascontiguousarray(x[b, half * SC : (half + 1) * SC].T.astype(bf16))
        maps.append(
            {
                "xtk": xtk,
                "xtq": xtq_b[b],
                "wqv": wqv,
                "wkk": wkk,
                "mb": mb_b[b],
                "idb": idb,
                "idf": idf,
            }
        )
    return maps, (J, nfull)


def kernel(x, mask, Wk, Wq, Wv):
    from concourse.bass_utils import run_bass_kernel_spmd

    maps, key = _in_maps(x, mask, Wk, Wq, Wv)
    if key not in _CACHE:
        _CACHE[key] = _build(*key)
    nc = _CACHE[key]
    br = run_bass_kernel_spmd(nc, maps, list(range(N_CORES)))
    out = np.empty((B, S, D), dtype=np.float32)
    for c in range(N_CORES):
        b, half = c // 2, c % 2
        out[b, half * SC : (half + 1) * SC, :] = br.results[c]["out"]
    return out
